# revision 15
# baseline (speedup 1.0000x reference)
"""Trainium2 Bass kernel for nn_ComplexConv2Deffangle — fp8 DoubleRow version.

Reference computation (per batch b):
  xr = x[b,0] (rot plane), xa = x[b,1] (mag plane), both [C=64, 64, 64]
  w1g = w1^2/sum(w1^2); w2g = w2^2/sum(w2^2)        (global-normalized)
  w1r = w1^2/rowsum;    w2r = w2^2/rowsum           (row-normalized)
  out_rot[o,ox,oy] = sum_{c,k} w2g[o,c]*w1g[c,k] * xr[c,ox+ki,oy+kj]
  out_abs[o,ox,oy] = exp( sum_{c,k} w2r[o,c]*w1r[c,k] * ln(xa+eps)[c,ox+ki,oy+kj] )

Each branch is a fused 3x3 conv with rank-structured weights
W_k[c,o] = w1n[c,k]*w2n[o,c] (precomputed on host).  The fp16 baseline
computed it as 9 accumulating 64-contraction matmuls per output tile
(1 col/cycle, both branches row-tiled on PE halves): ~58us of matmul
stream.  This version uses fp8 e4m3 with perf_mode=DoubleRow: each
matmul contracts TWO of the nine 3x3 shifts at once (64 channels x 2
shifts = 128 virtual rows on 64 physical partitions), so only 5 passes
per tile.  The pair of shifted conv windows is expressed as a 3D moving
access pattern [64 part, 2 (pair, step=delta), N] over the same input
plane — no data duplication.  Output rows are computed 64 wide (62
valid + 2 wrap-garbage cols, discarded at drain) so the window is
contiguous and the AP stays 3D.

Host precomputes ln(xa+eps) and converts both planes to fp8 e4m3
(ml_dtypes.float8_e4m3 == TRN FP8_EXP4, max 240), so the device does no
Ln at all.  Weights are pre-scaled by powers of two into fp8 range and
descaled at drain (DVE multiply for rot; folded into Exp's scale for
abs) — power-of-2 scaling is exact in fp8.

Outputs drain into per-(batch,branch) SBUF buffers and ship as 983KB
DMAs; rot outputs on the scalar HWDGE ring, abs outputs + inputs on the
sync ring (two physical HWDGE rings).

Sharding: pure data parallel over batch (32 -> 4 per core x 8 cores).
"""

import numpy as np

KH = KW = 3
EPS = 1e-6
B_FULL = 32
N_CORES = 8
BPC = B_FULL // N_CORES  # 4 batches per core
C, H, W = 64, 64, 64
O = 128
OX = OY = 62
RPT = 8  # output rows per L-tile (8*64=512 fp32 = one psum bank)
N_LT = (OX + RPT - 1) // RPT  # 8 (7 full + 1 of 6 rows)
NPASS = 5  # ceil(9/2) DoubleRow passes; slot 9 is zero-weighted
PADF = H * W  # xt free size (4D window APs never read past the plane)
S_ROT = 2.0**25  # rot weight pre-scale (host) / drain descale (device)
S_ABS = 2.0**8  # abs weight pre-scale / folded into Exp's scale param

_CACHE = {}


def _flat(k):
    return (k // KW) * W + (k % KW)


# DoubleRow pairing of the nine 3x3 shifts (k = 3*i + j, window offset
# _flat(k) = 64*i + j).  HW constraint (measured): the moving-operand pair
# stride must be EVEN — delta=1 hangs the PE, delta in {0,2,62,64} works.
# So pair even-offset shifts together and odd with odd:
#   {0,2} {64,66} {128,130} (delta=2), {1,65} (delta=64), {129,dup} (delta=0,
#   second slot zero-weighted).
PAIRS = [(0, 2), (3, 5), (6, 8), (1, 4), (7, 7)]


def _build_bass():
    import concourse.mybir as mybir
    import concourse.tile as tile
    from concourse import bacc
    from concourse.bass import AP

    f32 = mybir.dt.float32
    f16 = mybir.dt.float16
    f8 = mybir.dt.float8e4
    AF = mybir.ActivationFunctionType
    DR = mybir.MatmulPerfMode.DoubleRow

    nc = bacc.Bacc()
    x = nc.dram_tensor("x", [BPC, 2 * C, H * W], f8, kind="ExternalInput")
    wb = nc.dram_tensor("wb", [128, NPASS, 2, O], f8, kind="ExternalInput")
    out = nc.dram_tensor("out", [BPC, 2, O, OX, OY], f16, kind="ExternalOutput")

    with tile.TileContext(nc) as tc:
        N_WARM = 5

        with (
            tc.tile_pool(name="wpool", bufs=1) as wpool,
            tc.tile_pool(name="xpool", bufs=BPC) as xpool,
            tc.tile_pool(name="opool", bufs=4) as opool,
            tc.tile_pool(name="pspool", bufs=1, space="PSUM") as pspool,
        ):
            wsb = wpool.tile([128, NPASS, 2, O], f8, name="wsb")
            # weights first on the sync ring (scalar's ring starts ~1.7us
            # later because of the ACT table load): small, lands ~8.9us
            nc.sync.dma_start(wsb[:], wb[:, :, :, :])
            zero_t = wpool.tile([128, 1], f32, name="zero_t")
            scratch1 = wpool.tile([128, 1], f32, name="scratch1")
            nc.vector.memset(zero_t[:], 0.0)
            # dummy 1-element Exp: forces the exp ACT table load to happen
            # in parallel with input DMA instead of at the first drain
            nc.scalar.activation(scratch1[:], zero_t[:], AF.Exp)

            # HAM warm-up: dependency-free full-array matmuls bridge the
            # input-DMA window and get the PE clock gate to 2.4GHz before
            # the real matmul stream starts.
            warm_sb = wpool.tile([128, 512], f16, name="warm_sb")
            nc.vector.memset(warm_sb[:], 0.0)
            ps_warm = pspool.tile(
                [128, RPT, OY], f32, name="ps_warm", tag="ps_rot", bufs=4
            )
            for _ in range(N_WARM):
                nc.tensor.matmul(
                    ps_warm[:, :, :],
                    lhsT=warm_sb[:, 0:128],
                    rhs=warm_sb[:, 0 : RPT * OY],
                    start=True,
                    stop=True,
                )

            # Front-load all input DMAs on the sync queue (rot plane ->
            # partitions 0-63, ln(mag) plane -> 64-127, fp8).  Batch 0 is
            # split into 16-row chunks so the first L-tiles' matmuls can
            # start as soon as the first ~128KB lands instead of waiting
            # for the full plane.
            xts = []
            for b in range(BPC):
                xt = xpool.tile([128, PADF], f8, name="xt", tag="xt")
                if b == 0:
                    for r in range(0, H, 16):
                        nc.sync.dma_start(
                            xt[:, r * W : (r + 16) * W],
                            x[b][:, r * W : (r + 16) * W],
                        )
                else:
                    nc.sync.dma_start(xt[:, : H * W], x[b])
                xts.append(xt)

            def pair_rhs(xt, base_part, r0, p, nrows):
                """Moving AP [64, 2, nrows, OY]: the two shifted conv
                windows of pass p as a strided pair dim (step = window
                offset delta, even by HW requirement), rows/cols as a 4D
                window so no garbage columns are computed."""
                k0, k1 = PAIRS[p]
                f0, f1 = _flat(k0), _flat(k1)
                a = xt[:]
                return AP(
                    tensor=a.tensor,
                    offset=base_part * PADF + r0 * W + f0,
                    ap=[[PADF, 64], [f1 - f0, 2], [W, nrows], [1, OY]],
                )

            for b in range(BPC):
                xt = xts[b]
                o_rot = opool.tile([128, OX, OY], f16, name="o_rot", tag="o_rot")
                o_abs = opool.tile([128, OX, OY], f16, name="o_abs", tag="o_abs")
                for lt in range(N_LT):
                    r0 = lt * RPT
                    nrows = min(RPT, OX - r0)
                    ps_rot = pspool.tile(
                        [128, RPT, OY], f32, name="ps_rot", tag="ps_rot", bufs=4
                    )
                    ps_abs = pspool.tile(
                        [128, RPT, OY], f32, name="ps_abs", tag="ps_abs", bufs=4
                    )
                    for p in range(NPASS):
                        st = p == 0
                        sp = p == NPASS - 1
                        # rot branch on PE rows 0-63
                        nc.tensor.matmul(
                            ps_rot[:, :nrows, :],
                            lhsT=wsb[0:64, p, :, :],
                            rhs=pair_rhs(xt, 0, r0, p, nrows),
                            start=st,
                            stop=sp,
                            perf_mode=DR,
                        )
                        # abs branch on PE rows 64-127
                        nc.tensor.matmul(
                            ps_abs[:, :nrows, :],
                            lhsT=wsb[64:128, p, :, :],
                            rhs=pair_rhs(xt, 64, r0, p, nrows),
                            start=st,
                            stop=sp,
                            perf_mode=DR,
                        )
                    # drain into the per-batch output buffers (cols 62-63
                    # are window-wrap garbage, dropped here)
                    nc.vector.tensor_scalar_mul(
                        o_rot[:, r0 : r0 + nrows, :],
                        ps_rot[:, :nrows, :],
                        1.0 / S_ROT,
                    )
                    nc.scalar.activation(
                        o_abs[:, r0 : r0 + nrows, :],
                        ps_abs[:, :nrows, :],
                        AF.Exp,
                        scale=1.0 / S_ABS,
                    )
                    # ship finished rows in chunks so the final chunk after
                    # the last drain is small — the batch-3 output tail is
                    # HBM-bandwidth-bound otherwise.  rot on the scalar
                    # HWDGE ring, abs on sync: the two branches' outputs
                    # stream in parallel.  Chunks shrink toward the end
                    # (2,2,2,1,1 L-tiles) to minimize the post-stream tail.
                    if lt in (1, 3, 5, 6, 7):
                        rc = r0 - RPT if lt in (1, 3, 5) else r0
                        rce = r0 + nrows
                        nc.scalar.dma_start(
                            out[b, 0][:, rc:rce, :], o_rot[:, rc:rce, :]
                        )
                        nc.sync.dma_start(
                            out[b, 1][:, rc:rce, :], o_abs[:, rc:rce, :]
                        )
    nc.finalize()
    return nc


def _host_inputs(x):
    """[B, 2, C, H, W] f32 -> per-core [BPC, 128, H*W] fp8 planes:
    partitions 0-63 = rot plane, 64-127 = ln(mag + eps)."""
    import ml_dtypes

    x = np.asarray(x, np.float32)
    xr = x[:, 0]
    la = np.log(x[:, 1] + EPS)
    xs = np.concatenate([xr, la], axis=1)  # [B, 2C, H, W]
    xs = xs.reshape(N_CORES, BPC, 2 * C, H * W)
    return np.ascontiguousarray(xs.astype(ml_dtypes.float8_e4m3))


def _host_weights(w1, w2):
    """Pair weight tensor [128, NPASS, 2, O] fp8: partitions 0-63 hold the
    rot-branch (global-norm) weights scaled by S_ROT, 64-127 the abs-branch
    (row-norm) weights scaled by S_ABS.  Pass p slot s = shift k=2p+s
    (slot 9 zero)."""
    import ml_dtypes

    w1 = np.asarray(w1, np.float32)
    w2 = np.asarray(w2, np.float32)
    w1s = w1 * w1
    w2s = w2 * w2
    w1_glob = w1s / w1s.sum()
    w2_glob = w2s / w2s.sum()
    w1_row = w1s / w1s.sum(axis=1, keepdims=True)
    w2_row = w2s / w2s.sum(axis=1, keepdims=True)
    # full [c, k, o] = w1n[c, k] * w2n[o, c], distributed into the
    # even-delta pass pairs (dup slot of the last pass zero-weighted)
    rot = S_ROT * (w1_glob[:, :, None] * w2_glob.T[:, None, :])
    ab = S_ABS * (w1_row[:, :, None] * w2_row.T[:, None, :])
    wb = np.zeros((128, NPASS, 2, O), np.float32)
    for p, (k0, k1) in enumerate(PAIRS):
        wb[0:64, p, 0] = rot[:, k0]
        wb[64:128, p, 0] = ab[:, k0]
        if k1 != k0:
            wb[0:64, p, 1] = rot[:, k1]
            wb[64:128, p, 1] = ab[:, k1]
    wb = np.clip(wb, -240.0, 240.0)
    return np.ascontiguousarray(wb.astype(ml_dtypes.float8_e4m3))


def _ensure_ntff_hook():
    """The slim agent image lacks antenv.axon_hooks; recreate it so
    run_bass_kernel_spmd(trace=True) can capture NTFF profiles."""
    import sys
    import types

    if "antenv.axon_hooks" in sys.modules:
        return
    import antenv  # noqa: F401

    mod = types.ModuleType("antenv.axon_hooks")
    state = {"hook": None}
    mod.set_axon_ntff_profile_hook = lambda h: state.__setitem__("hook", h)
    mod.get_axon_ntff_profile_hook = lambda: state["hook"]
    sys.modules["antenv.axon_hooks"] = mod
    try:
        from trn_agent_boot.trn_boot import _ntff_profile_via_ctypes

        mod.set_axon_ntff_profile_hook(
            _ntff_profile_via_ctypes("/opt/axon/libaxon_pjrt.so")
        )
    except Exception:
        pass


def kernel(x, w1, w2, _trace=False):
    if _trace:
        _ensure_ntff_hook()
    from concourse.bass_utils import run_bass_kernel_spmd

    xs = _host_inputs(x)
    wbn = _host_weights(w1, w2)

    if "nc" not in _CACHE:
        _CACHE["nc"] = _build_bass()
    nc = _CACHE["nc"]

    in_maps = [{"x": xs[i], "wb": wbn} for i in range(N_CORES)]
    res = run_bass_kernel_spmd(
        nc, in_maps, core_ids=list(range(N_CORES)), trace=_trace
    )
    _CACHE["last_result"] = res
    outs = np.stack([r["out"] for r in res.results])  # [8, 4, 2, O, OX, OY] f16
    return outs.reshape(B_FULL, 2, O, OX, OY).astype(np.float32)
